# revision 77
# baseline (speedup 1.0000x reference)
"""RWKV-style block (nn_Block_83056077570124) on 8 Trainium2 NeuronCores.

Data-parallel over batch: one batch element per core, no collectives.
415514 ns (baseline) -> 253169 ns (CoreSim cost model), HW rel err ~1e-2.

Design:
  - fp8e4m3 + DoubleRow matmuls (2 K-planes per instruction) for k/v/r,
    Wo and Wr_ffn: weights host-prescaled x32 and pair-interleaved
    ([128,2,N] APs); the 1/32 comes out free in the psum drains (exp
    scale, copy scale, or folded into the gate reciprocal). Wk_ffn/Wv_ffn
    stay fp16: their fp8 error (relu^2 doubles it) measured over the 2e-2
    gate. Everything else fp16 off the f32 residual rows; psum fp32.
  - shift-free WKV: kexp' = e^(k+e^td) = kexp/a is produced by the k-psum
    drain (bias=e^td), so wkv[t] = (a*ef-1)*kv'[t] + S'[t] where S' is the
    running scan - no t-shifted adds, no column fixups.
  - sigmoid via exp: sig(r)*q = q/(wk*(1+e^-r)) with a single reciprocal;
    LN rstd via exp(-0.5*ln(var+eps)) -> every ACT function ({Copy, Exp,
    Ln, Relu, Square}) lives in one activation table (preloaded ATL id),
    zero table thrashing.
  - engine placement under HW ISA limits (Pool: no STT/scan/psum;
    DVE: no TT-divide): scans/STT/recip on DVE, muls/tensor_scalar on
    Pool, drains+accums on ACT, LN stats alternating ACT (accum) / DVE
    (bn_stats), x loads split SP/Pool half-rows.
  - transpose+mix in TCH-aligned halves so the first matmuls start after
    only 3 LN rows; k2 all materialized then kv2 accumulated across all
    32 h-blocks in one psum group per (och,row), tail staggered per row
    so the final drains overlap remaining matmuls.
"""
import sys

sys.path.insert(0, "/opt/trn_rl_repo")
import numpy as np

import concourse.bacc as bacc
import concourse.tile as tile
from concourse import mybir
from concourse.bass_utils import run_bass_kernel_spmd
from concourse.masks import make_identity

F32 = mybir.dt.float32
F32R = mybir.dt.float32r
F16 = mybir.dt.float16
F8 = mybir.dt.float8e4
AL = mybir.AluOpType
AF = mybir.ActivationFunctionType
DR = mybir.MatmulPerfMode.DoubleRow
W8S = 32.0   # host prescale for fp8 weights, undone at psum drain

B, T, C, H = 8, 768, 1024, 4096
NT = T // 128    # 6 row blocks (t on partitions)
NC = C // 128    # 8 channel blocks
NH = H // 128    # 32 ffn hidden blocks
TCH = [(0, 384), (384, 384)]    # t chunks for [o,t]-orientation psums
OCH = [(0, 512), (512, 512)]    # o chunks for [t,o]-orientation psums

_CACHE: dict = {}


def _build():
    nc = bacc.Bacc(trn_type="TRN2")

    x_d = nc.declare_dram_parameter("x", [T, C], F32, isOutput=False)
    # fp8 DoubleRow pair-slabs: arr[o*128+p, cp*256+i*128+j] =
    #   W[o*128+j, (2cp+i)*128+p] * W8S
    wkc_d = nc.declare_dram_parameter("wkc", [C, C], F8, isOutput=False)
    wvc_d = nc.declare_dram_parameter("wvc", [C, C], F8, isOutput=False)
    wrc_d = nc.declare_dram_parameter("wrc", [C, C], F8, isOutput=False)
    # col-slab layout f16: arr[o*128+p, ci*128+j] = W[o*128+j, ci*128+p]
    wkfc_d = nc.declare_dram_parameter("wkfc", [H, C], F16, isOutput=False)
    # fp8 DoubleRow pair-rows: arr[cp*128+p, i*C+j] = W.T[(2cp+i)*128+p, j] * W8S
    wor_d = nc.declare_dram_parameter("wor", [C // 2, 2 * C], F8, isOutput=False)
    wvfr_d = nc.declare_dram_parameter("wvfr", [H, C], F16, isOutput=False)
    wrfr_d = nc.declare_dram_parameter("wrfr", [C // 2, 2 * C], F8, isOutput=False)
    # packed per-channel consts [128, 32]: [tma | tmf | td | tf], col j within
    # each group = channel block j
    cst_d = nc.declare_dram_parameter("cst", [128, 4 * NC], F32, isOutput=False)
    out_d = nc.declare_dram_parameter("out", [T, C], F32, isOutput=True)

    # every ACT func used ({Copy, Exp, Ln, Relu, Square}) lives in one
    # activation table; preload it so the ATL pass never has to swap.
    from concourse.hw_specs import get_activation_tables
    _need = {AF.Copy, AF.Exp, AF.Ln, AF.Relu, AF.Square}
    _atl_id = next(i for i, (_, funcs) in enumerate(get_activation_tables(nc.m.arch).items())
                   if _need <= funcs)

    with tile.TileContext(nc) as tc, nc.allow_low_precision(reason="fp16 kernel"):
        atl = mybir.InstLoadActFuncSet(
            name=nc.get_next_instruction_name(), ins=[], outs=[],
            act_func_set_id=_atl_id)
        atl.engine = mybir.EngineType.Activation
        nc._add_instruction(atl)
        with (
            tc.tile_pool(name="const", bufs=1) as cstp,
            tc.tile_pool(name="small", bufs=1) as smp,
            tc.tile_pool(name="rows", bufs=1) as rowp,
            tc.tile_pool(name="junkp", bufs=1) as junkp,
            tc.tile_pool(name="xnp", bufs=8) as xnp,
            tc.tile_pool(name="xmp", bufs=8) as xmp,
            tc.tile_pool(name="big16", bufs=16) as bigp,
            tc.tile_pool(name="tmp16", bufs=2) as tmpp,
            tc.tile_pool(name="slab", bufs=3) as slabp,
            tc.tile_pool(name="wrow", bufs=6) as wrowp,
            tc.tile_pool(name="psp", bufs=6, space="PSUM") as psp,
            tc.tile_pool(name="psp2", bufs=2, space="PSUM") as psp2,
        ):
            ident = cstp.tile([128, 128], F32, tag="ident")
            make_identity(nc, ident[:])
            cinv = cstp.tile([128, 1], F32, tag="cinv")
            nc.gpsimd.memset(cinv[:], 1.0 / C)
            eps_t = cstp.tile([128, 1], F32, tag="eps")
            nc.gpsimd.memset(eps_t[:], 1e-5)
            w8inv = cstp.tile([128, 1], F32, tag="w8inv")
            nc.gpsimd.memset(w8inv[:], 1.0 / W8S)

            # ---- packed consts (one DMA)
            cst_t = cstp.tile([128, 4 * NC], F32, tag="cst")
            nc.gpsimd.dma_start(out=cst_t[:], in_=cst_d[:, :])
            om_t = cstp.tile([128, 2 * NC], F32, tag="om")
            nc.scalar.activation(om_t[:], cst_t[:, 0:2 * NC], AF.Copy, bias=1.0, scale=-1.0)
            ed_t = cstp.tile([128, NC], F32, tag="ed")
            nc.scalar.activation(ed_t[:], cst_t[:, 2 * NC:3 * NC], AF.Exp)  # e^td
            a_t = cstp.tile([128, NC], F32, tag="a")
            nc.scalar.activation(a_t[:], ed_t[:], AF.Exp, scale=-1.0)       # e^-e^td
            ef_t = cstp.tile([128, NC], F32, tag="ef")
            nc.scalar.activation(ef_t[:], cst_t[:, 3 * NC:4 * NC], AF.Exp)  # e^tf
            c1_t = cstp.tile([128, NC], F32, tag="c1")
            nc.vector.tensor_mul(c1_t[:], a_t[:], ef_t[:])
            nc.vector.tensor_scalar_add(c1_t[:], c1_t[:], -1.0)   # a*ef - 1
            tma_c = lambda j: cst_t[:, j:j + 1]
            tmf_c = lambda j: cst_t[:, NC + j:NC + j + 1]
            omta_c = lambda j: om_t[:, j:j + 1]
            omtf_c = lambda j: om_t[:, NC + j:NC + j + 1]

            # ---- x rows: half-row DMAs, left halves on SP, right halves on
            # Pool, so row i lands at ~0.8*(i+1) us and ACT stays free
            xres = []
            for i in range(NT):
                xi = rowp.tile([128, C], F32, tag=f"xres{i}")
                nc.sync.dma_start(out=xi[:, 0:512], in_=x_d[i * 128:(i + 1) * 128, 0:512])
                nc.gpsimd.dma_start(out=xi[:, 512:1024], in_=x_d[i * 128:(i + 1) * 128, 512:1024])
                xres.append(xi)

            def layer_norm_row(i, phase):
                """In-place LN of xres[i]. Stats alternate between ACT
                (copy/square accums) and DVE (bn_stats) so prologue rows
                pipeline across both engines; rstd = exp(-.5*ln(var+eps));
                fused (x-mu)*rstd on DVE."""
                src = xres[i]
                if i in (0, 3):
                    mu_t = smp.tile([128, 1], F32, tag=f"mu{phase}_{i}")
                    varr_t = smp.tile([128, 1], F32, tag=f"var{phase}_{i}")
                    mu = mu_t[:]
                    varr = varr_t[:]
                    junk = junkp.tile([128, C], F32, tag="junk")
                    sm = smp.tile([128, 1], F32, tag=f"sm{phase}_{i}")
                    nc.scalar.activation(junk[:], src[:], AF.Copy, accum_out=sm[:])
                    junk2 = junkp.tile([128, C], F32, tag="junk")
                    ssq = smp.tile([128, 1], F32, tag=f"ssq{phase}_{i}")
                    nc.scalar.activation(junk2[:], src[:], AF.Square, accum_out=ssq[:])
                    nc.vector.tensor_scalar_mul(mu, sm[:], 1.0 / C)
                    m2 = smp.tile([128, 1], F32, tag=f"m2{phase}_{i}")
                    nc.vector.tensor_mul(m2[:], mu, mu)
                    nc.vector.scalar_tensor_tensor(
                        out=varr, in0=ssq[:], scalar=cinv[:], in1=m2[:],
                        op0=AL.mult, op1=AL.subtract)
                else:
                    stats = smp.tile([128, 12], F32, tag=f"bns{phase}_{i}")
                    nc.vector.bn_stats(out=stats[:, 0:6], in_=src[:, 0:512])
                    nc.vector.bn_stats(out=stats[:, 6:12], in_=src[:, 512:1024])
                    mv = smp.tile([128, 2], F32, tag=f"mv{phase}_{i}")
                    nc.vector.bn_aggr(out=mv[:], in_=stats[:])
                    mu = mv[:, 0:1]
                    varr = mv[:, 1:2]
                lnv = smp.tile([128, 1], F32, tag=f"lnv{phase}_{i}")
                nc.scalar.activation(lnv[:], varr, AF.Ln, bias=eps_t[:])
                rstd = smp.tile([128, 1], F32, tag=f"rstd{phase}_{i}")
                nc.scalar.activation(rstd[:], lnv[:], AF.Exp, scale=-0.5)
                ts_eng = nc.vector if i < 3 else nc.gpsimd
                ts_eng.tensor_scalar(
                    out=src[:], in0=src[:], scalar1=mu, scalar2=rstd[:],
                    op0=AL.subtract, op1=AL.mult)

            # ---- transpose + mix in TCH-aligned halves: half 0 (rows 0-2 ->
            # cols 0:384) unblocks the tch=0 matmul groups after only 3 LN rows
            def transpose_half(j, h, xnT):
                ps = psp2.tile([128, 512], F32, tag="ps2", name="tps")
                for idx, i in enumerate(range(3 * h, 3 * h + 3)):
                    nc.tensor.transpose(
                        ps[:, idx * 128:(idx + 1) * 128],
                        xres[i][:, j * 128:(j + 1) * 128],
                        ident[:])
                if (h + j) % 2 == 0:
                    nc.vector.tensor_copy(xnT[:, h * 384:h * 384 + 384], ps[:, 0:384])
                else:
                    nc.scalar.copy(xnT[:, h * 384:h * 384 + 384], ps[:, 0:384])

            def mix_half(j, h, xnT, xm, tm_c, omtm_c):
                # xm = tm*xn + omtm*shift(xn); STT is not legal on Pool, so
                # Pool does ts_ptr + tensor_add with a temp
                c0, c1 = (0, 384) if h == 0 else (384, 768)
                nc.vector.tensor_scalar_mul(
                    xm[:, c0:c1], xnT[:, c0:c1], tm_c(j))
                s0 = max(c0, 1)
                mt = tmpp.tile([128, 384], F16, tag="mixt", name="mixt")
                nc.gpsimd.tensor_scalar_mul(
                    mt[:, 0:c1 - s0], xnT[:, s0 - 1:c1 - 1], omtm_c(j))
                nc.gpsimd.tensor_add(
                    xm[:, s0:c1], xm[:, s0:c1], mt[:, 0:c1 - s0])

            def mix_half_f8(j, h, xnT, xm8pair, tm_c, omtm_c):
                # same mix, but summed on DVE straight into the f8 pair plane
                c0, c1 = (0, 384) if h == 0 else (384, 768)
                s0 = max(c0, 1)
                t1 = tmpp.tile([128, 384], F16, tag="mixa", name="mixa")
                nc.vector.tensor_scalar_mul(t1[:, 0:c1 - c0], xnT[:, c0:c1], tm_c(j))
                t2 = tmpp.tile([128, 384], F16, tag="mixt", name="mixt")
                nc.gpsimd.tensor_scalar_mul(
                    t2[:, 0:c1 - s0], xnT[:, s0 - 1:c1 - 1], omtm_c(j))
                dst = xm8pair[:, j % 2, :]
                if h == 0:
                    nc.vector.tensor_copy(dst[:, 0:1], t1[:, 0:1])
                nc.gpsimd.tensor_add(
                    dst[:, s0:c1], t1[:, s0 - c0:c1 - c0], t2[:, 0:c1 - s0])

            def make_xms(tm_c, omtm_c, name, xm8=None):
                xnTs = [xnp.tile([128, T], F16, tag="xnT", name=f"xnT{name}{j}")
                        for j in range(NC)]
                xms = None
                if xm8 is None:
                    xms = [xmp.tile([128, T], F16, tag="xm", name=f"xm{name}{j}")
                           for j in range(NC)]
                for h in range(2):
                    for j in range(NC):
                        transpose_half(j, h, xnTs[j])
                        if xm8 is None:
                            mix_half(j, h, xnTs[j], xms[j], tm_c, omtm_c)
                        else:
                            mix_half_f8(j, h, xnTs[j], xm8[j // 2], tm_c, omtm_c)
                return xms

            def load_slab(dram, o, engine, dtype=F16):
                if dtype is F8:
                    w = slabp.tile([128, NC, 128], F8, tag="slab8", name=f"slab8_{o}", bufs=4)
                else:
                    w = slabp.tile([128, C], F16, tag="slab", name=f"slab{o}")
                engine.dma_start(out=w[:], in_=dram[o * 128:(o + 1) * 128, :])
                return w

            def load_wrow(dram, r, engine, name, tag="wrow"):
                w = wrowp.tile([128, C], F16, tag=tag, name=f"{name}{r}")
                engine.dma_start(out=w[:], in_=dram[r * 128:(r + 1) * 128, :])
                return w

            def mm_ot(slab, moving, drain):
                """psum[o-coords, t] = sum_ci slab[:,ci]' . moving[ci][:,t]"""
                for (t0, tn) in TCH:
                    ps = psp.tile([128, 512], F32, tag="ps", name="ps")
                    for ci in range(NC):
                        nc.tensor.matmul(
                            ps[:, 0:tn],
                            slab[:, ci * 128:(ci + 1) * 128],
                            moving[ci][:, t0:t0 + tn],
                            start=(ci == 0), stop=(ci == NC - 1))
                    drain(slice(t0, t0 + tn), ps[:, 0:tn])

            def mm_ot8(slab8, xm8, drain):
                """fp8 DoubleRow variant: slab8 [128, NC, 128], xm8 pair tiles
                [128, 2, T]; psum[o-coords, t] over 4 K=256 pair-matmuls."""
                for (t0, tn) in TCH:
                    ps = psp.tile([128, 512], F32, tag="ps", name="ps")
                    for cp in range(NC // 2):
                        nc.tensor.matmul(
                            ps[:, 0:tn],
                            slab8[:, 2 * cp:2 * cp + 2, :],
                            xm8[cp][:, :, t0:t0 + tn],
                            start=(cp == 0), stop=(cp == NC // 2 - 1),
                            perf_mode=DR)
                    drain(slice(t0, t0 + tn), ps[:, 0:tn])

            # =================== LN1 + att mix (straight to f8 pairs) ==========
            for i in range(NT):
                layer_norm_row(i, 0)
            xm8_att = [xmp.tile([128, 2, T], F8, tag="xm8", name=f"xm8a{cp}")
                       for cp in range(NC // 2)]
            make_xms(tma_c, omta_c, "a", xm8=xm8_att)

            # =================== att: k/v/r (fp8 DR) + WKV per o-block =========
            # shift-free WKV: with kexp' = e^(k+td') = kexp/a (a = e^-e^td),
            # S'[t] = a S'[t-1] + kexp'[t]*v[t]:
            #   wkv[t] = (a*ef-1)*kv'[t] + S'[t],  wk likewise -> no t-shifts
            rw8p = [xmp.tile([128, 2, T], F8, tag="rw8", name=f"rw8_{cp}", bufs=4)
                    for cp in range(NC // 2)]
            for o in range(NC):
                wk_s = load_slab(wkc_d, o, nc.sync, dtype=F8)
                wv_s = load_slab(wvc_d, o, nc.sync, dtype=F8)
                wr_s = load_slab(wrc_d, o, nc.sync, dtype=F8)

                edj = ed_t[:, o:o + 1]
                kexp = tmpp.tile([128, T], F16, tag="kexp", name=f"kexp{o}")
                mm_ot8(wk_s, xm8_att,
                       lambda ts, ps: nc.scalar.activation(
                           kexp[:, ts], ps, AF.Exp, scale=1.0 / W8S, bias=edj))
                v16 = tmpp.tile([128, T], F16, tag="v16", name=f"v16{o}")
                mm_ot8(wv_s, xm8_att,
                       lambda ts, ps: nc.scalar.activation(
                           v16[:, ts], ps, AF.Copy, scale=1.0 / W8S))
                # e^-r for the sigmoid-gate reciprocal
                emr = tmpp.tile([128, T], F16, tag="emr", name=f"emr{o}")
                mm_ot8(wr_s, xm8_att,
                       lambda ts, ps: nc.scalar.activation(
                           emr[:, ts], ps, AF.Exp, scale=-1.0 / W8S))

                aj = a_t[:, o:o + 1]
                c1j = c1_t[:, o:o + 1]
                ab = aj.broadcast_to([128, T])
                kv = tmpp.tile([128, T], F16, tag="kv", name=f"kv{o}")
                nc.gpsimd.tensor_mul(kv[:], kexp[:], v16[:])
                S = tmpp.tile([128, T], F16, tag="S", name=f"S{o}")
                nc.vector.tensor_tensor_scan(
                    out=S[:], data0=ab, data1=kv[:], initial=0.0,
                    op0=AL.mult, op1=AL.add)
                Sk = tmpp.tile([128, T], F16, tag="Sk", name=f"Sk{o}")
                nc.vector.tensor_tensor_scan(
                    out=Sk[:], data0=ab, data1=kexp[:], initial=0.0,
                    op0=AL.mult, op1=AL.add)
                wkv = tmpp.tile([128, T], F16, tag="wkv", name=f"wkv{o}")
                nc.vector.scalar_tensor_tensor(
                    out=wkv[:], in0=kv[:], scalar=c1j, in1=S[:],
                    op0=AL.mult, op1=AL.add)
                wk = tmpp.tile([128, T], F16, tag="wk", name=f"wk{o}")
                nc.gpsimd.tensor_scalar_mul(wk[:], kexp[:], c1j)
                nc.gpsimd.tensor_add(wk[:], wk[:], Sk[:])
                # rwkv = sig(r)*wkv/wk = wkv / (wk * (1 + e^-r))
                ope = tmpp.tile([128, T], F16, tag="ope", name=f"ope{o}", bufs=1)
                nc.scalar.activation(ope[:], emr[:], AF.Copy, bias=1.0)
                den = tmpp.tile([128, T], F32, tag="den", name=f"den{o}", bufs=1)
                nc.gpsimd.tensor_mul(den[:], wk[:], ope[:])
                rcp = tmpp.tile([128, T], F16, tag="rcp", name=f"rcp{o}")
                nc.vector.reciprocal(rcp[:], den[:])
                rw8 = rw8p[o // 2]
                nc.gpsimd.tensor_mul(rw8[:, o % 2, :], wkv[:], rcp[:])

            # =================== Wo (fp8 DR mat_to) + LN2 + ffn mix ============
            wo8 = []
            for cp in range(NC // 2):
                w = wrowp.tile([128, 2, C], F8, tag="wrf8", name=f"wo8_{cp}", bufs=4)
                nc.sync.dma_start(out=w[:], in_=wor_d[cp * 128:(cp + 1) * 128, :])
                wo8.append(w)
            for i in range(NT):
                tsl = slice(i * 128, (i + 1) * 128)
                for (o0, on) in OCH:
                    ps = psp.tile([128, 512], F32, tag="ps", name="ps")
                    for cp in range(NC // 2):
                        nc.tensor.matmul(
                            ps[:, 0:on],
                            rw8p[cp][:, :, tsl],
                            wo8[cp][:, :, o0:o0 + on],
                            start=(cp == 0), stop=(cp == NC // 2 - 1),
                            perf_mode=DR)
                    nc.vector.scalar_tensor_tensor(
                        out=xres[i][:, o0:o0 + on], in0=ps[:, 0:on], scalar=w8inv[:],
                        in1=xres[i][:, o0:o0 + on], op0=AL.mult, op1=AL.add)
                layer_norm_row(i, 1)
            xm_ffn = make_xms(tmf_c, omtf_c, "f")

            # =================== FFN k2 = relu(xm2 @ Wkf)^2 ===================
            k2 = []
            for ho in range(NH):
                wkf_s = load_slab(wkfc_d, ho, nc.sync)
                k2b = bigp.tile([128, T], F16, tag="big", name=f"k2_{ho}", bufs=32)

                def drain_k2(ts, ps, k2b=k2b):
                    tn = ps.shape[1]
                    kr = tmpp.tile([128, 384], F16, tag="kr", name="kr")
                    nc.scalar.activation(kr[:, 0:tn], ps, AF.Relu)
                    nc.vector.tensor_mul(k2b[:, ts], kr[:, 0:tn], kr[:, 0:tn])

                mm_ot(wkf_s, xm_ffn, drain_k2)
                k2.append(k2b)

            # =================== FFN r2 gate precompute (fp8 DR) ===============
            # rcp2[och][i] = 1/(1 + e^-r2): computed before kv2 so the final
            # kv2 drains are just mul+add+store.
            xmf8 = [xmp.tile([128, 2, T], F8, tag="xm8", name=f"xm8f{cp}")
                    for cp in range(NC // 2)]
            for cp in range(NC // 2):
                for i2 in range(2):
                    nc.gpsimd.tensor_copy(xmf8[cp][:, i2, :], xm_ffn[2 * cp + i2][:])
            wrf8 = []
            for cp in range(NC // 2):
                w = wrowp.tile([128, 2, C], F8, tag="wrf8", name=f"wrf8_{cp}", bufs=4)
                nc.sync.dma_start(out=w[:], in_=wrfr_d[cp * 128:(cp + 1) * 128, :])
                wrf8.append(w)
            ope2 = {}
            for oi, (o0, on) in enumerate(OCH):
                for i in range(NT):
                    tsl = slice(i * 128, (i + 1) * 128)
                    ps2 = psp2.tile([128, 512], F32, tag="ps2", name="ps2")
                    for cp in range(NC // 2):
                        nc.tensor.matmul(
                            ps2[:, 0:on],
                            xmf8[cp][:, :, tsl],
                            wrf8[cp][:, :, o0:o0 + on],
                            start=(cp == 0), stop=(cp == NC // 2 - 1),
                            perf_mode=DR)
                    emr2 = tmpp.tile([128, 512], F16, tag="emr2", name="emr2")
                    nc.scalar.activation(emr2[:, 0:on], ps2[:, 0:on], AF.Exp,
                                         scale=-1.0 / W8S)
                    op2 = tmpp.tile([128, 512], F32, tag="ope2", name="ope2", bufs=1)
                    nc.gpsimd.tensor_scalar_add(op2[:, 0:on], emr2[:, 0:on], 1.0)
                    rcp2 = tmpp.tile([128, 512], F16, tag="rcp2", name=f"rcp2_{oi}_{i}",
                                     bufs=12)
                    nc.vector.reciprocal(rcp2[:, 0:on], op2[:, 0:on])
                    ope2[(oi, i)] = rcp2

            # =================== FFN kv2 (fp8 DR), och-split, hp-outer =========
            for oi, (o0, on) in enumerate(OCH):
                osl = slice(o0, o0 + on)
                pss = [psp.tile([128, 512], F32, tag="ps", name=f"kv2ps{i}")
                       for i in range(NT)]
                # common part: hp-outer so wvf pair tiles stream; tail part
                # row-by-row so psum groups complete staggered and the
                # drains overlap the remaining matmuls
                HCUT = NH - 4
                wvf_tail = []
                for hi in range(NH):
                    eng = nc.sync if hi % 2 == 0 else nc.gpsimd
                    wvf = load_wrow(wvfr_d, hi, eng, f"wvf{o0}_")
                    if hi >= HCUT:
                        wvf_tail.append(wvf)
                        continue
                    for i in range(NT):
                        nc.tensor.matmul(
                            pss[i][:, 0:on],
                            k2[hi][:, i * 128:(i + 1) * 128],
                            wvf[:, o0:o0 + on],
                            start=(hi == 0), stop=False)
                for i in range(NT):
                    for hi in range(HCUT, NH):
                        nc.tensor.matmul(
                            pss[i][:, 0:on],
                            k2[hi][:, i * 128:(i + 1) * 128],
                            wvf_tail[hi - HCUT][:, o0:o0 + on],
                            start=False, stop=(hi == NH - 1))
                    gt = tmpp.tile([128, 512], F32, tag="gt", name="gt", bufs=1)
                    last = (oi == len(OCH) - 1 and i == NT - 1)
                    chunks = [(0, on // 2), (on // 2, on)] if last else [(0, on)]
                    for (c0, c1) in chunks:
                        nc.vector.tensor_mul(
                            gt[:, c0:c1], pss[i][:, c0:c1],
                            ope2[(oi, i)][:, c0:c1])
                        nc.vector.tensor_add(
                            xres[i][:, o0 + c0:o0 + c1], xres[i][:, o0 + c0:o0 + c1],
                            gt[:, c0:c1])
                        nc.sync.dma_start(
                            out=out_d[i * 128:(i + 1) * 128, o0 + c0:o0 + c1],
                            in_=xres[i][:, o0 + c0:o0 + c1])

    nc.compile()
    return nc


def _get_nc():
    if "nc" not in _CACHE:
        _CACHE["nc"] = _build()
    return _CACHE["nc"]


import ml_dtypes

NPF8 = ml_dtypes.float8_e4m3


def _col_slab(W):
    """W [Cout, Cin] -> arr[o*128+p, ci*128+j] = W[o*128+j, ci*128+p], f16."""
    Co, Ci = W.shape
    no, nci = Co // 128, Ci // 128
    return np.ascontiguousarray(
        W.reshape(no, 128, nci, 128).transpose(0, 3, 2, 1).reshape(Co, Ci)
        .astype(np.float16))


def _pair_slab(W):
    """fp8 DoubleRow pair-slab: arr[o*128+p, cp*256+i*128+j] =
    W[o*128+j, (2cp+i)*128+p] * W8S."""
    Co, Ci = W.shape
    A = (W * W8S).reshape(Co // 128, 128, Ci // 256, 2, 128).transpose(0, 4, 2, 3, 1)
    return np.ascontiguousarray(A.reshape(Co, Ci).astype(NPF8))


def _pair_rows(W):
    """fp8 DoubleRow pair-rows of W.T: arr[cp*128+p, i*Cout+j] =
    W.T[(2cp+i)*128+p, j] * W8S."""
    WT = W.T * W8S
    Ci, Co = WT.shape
    A = WT.reshape(Ci // 256, 2, 128, Co).transpose(0, 2, 1, 3)
    return np.ascontiguousarray(A.reshape(Ci // 2, 2 * Co).astype(NPF8))


def _pack8(v):
    return np.ascontiguousarray(
        np.asarray(v, np.float32).reshape(NC, 128).T)


def prepare_in_maps(inputs):
    f = np.ascontiguousarray
    g = np.asarray
    x = g(inputs["x"], np.float32)
    shared = {
        "wkc": _pair_slab(g(inputs["Wk_att"], np.float32)),
        "wvc": _pair_slab(g(inputs["Wv_att"], np.float32)),
        "wrc": _pair_slab(g(inputs["Wr_att"], np.float32)),
        "wkfc": _col_slab(g(inputs["Wk_ffn"], np.float32)),
        "wor": _pair_rows(g(inputs["Wo_att"], np.float32)),
        "wvfr": f(g(inputs["Wv_ffn"], np.float32).T.astype(np.float16)),
        "wrfr": _pair_rows(g(inputs["Wr_ffn"], np.float32)),
        "cst": np.ascontiguousarray(np.concatenate(
            [_pack8(inputs["tm_att"]), _pack8(inputs["tm_ffn"]),
             _pack8(inputs["time_decay"]), _pack8(inputs["time_first"])], axis=1)),
    }
    return [{**shared, "x": f(x[b])} for b in range(B)]


def run_full(inputs, **run_kwargs):
    nc = _get_nc()
    in_maps = prepare_in_maps(inputs)
    res = run_bass_kernel_spmd(nc, in_maps, list(range(B)), **run_kwargs)
    out = np.stack([res.results[b]["out"] for b in range(B)]).astype(np.float32)
    return out, res


def kernel(**inputs) -> np.ndarray:
    out, _ = run_full(inputs)
    return out
